# revision 6
# baseline (speedup 1.0000x reference)
"""Trainium2 Bass kernel for nn_CubECLayr: Euler characteristic curves of
sublevel cubical complexes, batch-data-parallel over 8 NeuronCores.

Algorithm (per core, 24 images of 256x256):
  1. k = ceil(x / DT) per pixel (exact integer bin in fp16), via fused
     multiply + magic-number round on the vector engine.
  2. Vertex attribution: every cell (vertex/edge/square) of the cubical
     complex is anchored to its (value, index)-max vertex; the signed count
     of cells anchored at each pixel is an integer delta in [-3, 3] computed
     from neighbor comparisons in k-space.  Then
         ECC_s = sum_p delta_p * [k_p <= s]
     which is exactly V - E + Sq of the sublevel complex at threshold s.
  3. The 32 threshold reductions are split across TWO engines:
     - Vector (DVE): direct fused compare-multiply-accumulate
       (scalar_tensor_tensor) -> sum(delta * [k <= s]) in one pass.
     - Scalar (ACT): encode e = k + delta/16 (exact in f16).  For
       m = s + 0.5:  sum(relu(m-k)) - sum(relu(m-e)) = (1/16)*sum(delta*
       [k<=s]), each sum one activation+accumulate pass.
     A block-diagonal ones matmul reduces partitions -> per-image curves.

Layout: 3 chunks x 8 images; each image owns 16 partitions (16 rows each,
one halo row above/below via SBUF shift-DMAs, pad=1000 at image borders).
"""

import numpy as np

import concourse.bacc as bacc
import concourse.mybir as mybir
from concourse import tile
from concourse.bass_utils import run_bass_kernel_spmd

NCORES = 8
B, C, H, W = 64, 3, 256, 256
IMGS = (B // NCORES) * C          # 24 images per core
CHUNK_IMGS = 8
NCHUNK = IMGS // CHUNK_IMGS       # 3
RB = 16                           # partitions per image
ROWS = H // RB                    # 16 own rows per partition
FD = ROWS * W                     # 4096 own pixels per partition
STEPS = 32
PAD = 1000.0                      # > any real bin; exact in fp16
MAGIC = 8388608.0                 # 2^23
HALF = float(np.float32(0.49999997))
F32 = mybir.dt.float32
F16 = mybir.dt.float16
Op = mybir.AluOpType
Act = mybir.ActivationFunctionType

# threshold split: vector does s in [0, V_THR) via one-pass STT; the scalar
# engine does s in [V_THR, 32) via relu-pair activations on (kt, et).
V_THR = 18
S_THR = STEPS - V_THR

_NC_CACHE = {}


def _build_nc():
    nc = bacc.Bacc(None, target_bir_lowering=False)
    x_in = nc.dram_tensor("x", [NCHUNK * 128, FD], F32, kind="ExternalInput")
    bd_in = nc.dram_tensor("bd", [128, NCHUNK * IMGS], F32, kind="ExternalInput")
    out = nc.dram_tensor("out", [IMGS, STEPS], F32, kind="ExternalOutput")

    HW2 = (ROWS + 2) * W          # kt/khe width (own + 2 halo rows)
    HW1 = (ROWS + 1) * W          # rv/ut width

    with tile.TileContext(nc) as tc:
        with (
            tc.tile_pool(name="xp", bufs=2) as xp,
            tc.tile_pool(name="wp", bufs=1) as wp,
            tc.tile_pool(name="ap", bufs=2) as ap,
            tc.tile_pool(name="cst", bufs=1) as cst,
            tc.tile_pool(name="pp", bufs=1, space="PSUM") as pp,
        ):
            bdt = cst.tile([128, NCHUNK * IMGS], F32)
            nc.sync.dma_start(out=bdt[:], in_=bd_in[:])
            padt = cst.tile([CHUNK_IMGS, W], F16)
            nc.vector.memset(padt[:], PAD)
            # per-threshold bias values m = s + 0.5 for the scalar engine
            biast = cst.tile([128, S_THR], F32)
            for i in range(S_THR):
                nc.gpsimd.memset(biast[:, i:i + 1], V_THR + i + 0.5)
            psum = pp.tile([IMGS, STEPS], F32)

            for c in range(NCHUNK):
                xt = xp.tile([128, FD], F32, tag="xt")
                nc.sync.dma_start(out=xt[:], in_=x_in[c * 128:(c + 1) * 128, :])

                # --- bins: k = round(x*31 + (0.5 - eps)) == ceil(x/DT) ---
                nc.vector.tensor_scalar(
                    out=xt[:], in0=xt[:], scalar1=31.0, scalar2=HALF,
                    op0=Op.mult, op1=Op.add)
                # kt rows: 0 = top halo, 1..16 own, 17 = bottom halo (flat cols)
                kt = wp.tile([128, HW2], F16, tag="kt")
                nc.vector.tensor_scalar(
                    out=kt[:, W:W + FD], in0=xt[:], scalar1=MAGIC, scalar2=-MAGIC,
                    op0=Op.add, op1=Op.add)
                # halo exchange between partitions (same image), pad at borders
                nc.vector.memset(kt[:, 0:W], PAD)
                nc.vector.memset(kt[:, FD + W:FD + 2 * W], PAD)
                nc.sync.dma_start(out=kt[1:128, 0:W], in_=kt[0:127, FD:FD + W])
                nc.sync.dma_start(out=kt[0:127, FD + W:FD + 2 * W],
                                  in_=kt[1:128, W:2 * W])
                ktop = kt[:, 0:W].rearrange("(a b) w -> a b w", b=RB)
                nc.sync.dma_start(out=ktop[:, 0, :], in_=padt[:])
                kbot = kt[:, FD + W:FD + 2 * W].rearrange("(a b) w -> a b w", b=RB)
                nc.sync.dma_start(out=kbot[:, RB - 1, :], in_=padt[:])
                kown = kt[:, W:W + FD]

                # --- neighbor comparisons (k-space) ---
                # rh[r, j] = [k(r, j+1) >= k(r, j)], own rows, j = 0..254
                # (col 255 crosses rows; harmless, later masked via t zeroing)
                rh = wp.tile([128, FD], F16, tag="rh")
                nc.vector.tensor_tensor(
                    out=rh[:], in0=kt[:, W + 1:W + FD + 1], in1=kown,
                    op=Op.is_ge)
                # rv[t, j] = [k(row t+1) >= k(row t)], t = 0..16 (17 rows)
                rv = wp.tile([128, HW1], F16, tag="rv")
                nc.vector.tensor_tensor(
                    out=rv[:], in0=kt[:, W:], in1=kt[:, 0:HW1], op=Op.is_ge)
                # khe[r, j] = max(k(r, j), k(r, j+1)), rows 0..17
                khe = wp.tile([128, HW2], F16, tag="khe")
                nc.vector.tensor_tensor(
                    out=khe[:, 0:HW2 - 1], in0=kt[:, 0:HW2 - 1],
                    in1=kt[:, 1:HW2], op=Op.max)
                nc.vector.memset(khe[:, HW2 - 1:HW2], PAD)
                # u[t, j] = [khe(row t+1, j) >= khe(row t, j)], t = 0..16
                ut = wp.tile([128, HW1], F16, tag="ut")
                nc.vector.tensor_tensor(
                    out=ut[:], in0=khe[:, W:], in1=khe[:, 0:HW1], op=Op.is_ge)
                # Cc[r, j] = u(r) - u(r-1) for own rows r (u rows 1..16 - 0..15)
                cc = wp.tile([128, FD], F16, tag="cc")
                nc.vector.tensor_tensor(
                    out=cc[:], in0=ut[:, W:], in1=ut[:, 0:FD], op=Op.subtract)
                # zero col 255 of each row (cross-row garbage in rh/cc)
                cc3 = cc[:].rearrange("p (r w) -> p r w", w=W)
                nc.vector.memset(cc3[:, :, W - 1:W], 0.0)
                # t = rh * Cc
                tt = wp.tile([128, FD], F16, tag="tt")
                nc.vector.tensor_tensor(out=tt[:], in0=rh[:], in1=cc[:], op=Op.mult)

                # --- delta assembly ---
                # delta = rv(below) - rv(above) + t - shift1(t) - Cc
                dl = wp.tile([128, FD], F16, tag="dl")
                nc.vector.tensor_tensor(
                    out=dl[:], in0=rv[:, W:], in1=rv[:, 0:FD], op=Op.subtract)
                nc.vector.tensor_tensor(out=dl[:], in0=dl[:], in1=tt[:], op=Op.add)
                nc.vector.tensor_tensor(
                    out=dl[:], in0=dl[:], in1=cc[:], op=Op.subtract)
                nc.vector.tensor_tensor(
                    out=dl[:, 1:FD], in0=dl[:, 1:FD], in1=tt[:, 0:FD - 1],
                    op=Op.subtract)

                # --- e = k + delta/16 (exact in f16), for the scalar engine ---
                dl16 = wp.tile([128, FD], F16, tag="dl16")
                nc.scalar.activation(
                    out=dl16[:], in_=dl[:], func=Act.Copy, bias=0.0,
                    scale=1.0 / 16.0)
                et = wp.tile([128, FD], F16, tag="et")
                nc.vector.tensor_tensor(out=et[:], in0=kown, in1=dl16[:], op=Op.add)

                # --- 32 threshold reductions, split across 2 engines ---
                accV = ap.tile([128, V_THR], F32, tag="accv")
                accSA = ap.tile([128, S_THR], F32, tag="accsa")
                accSB = ap.tile([128, S_THR], F32, tag="accsb")
                wm_v = wp.tile([128, FD], F16, tag="wmv")
                wm_s = wp.tile([128, FD], F16, tag="wms")
                for i in range(V_THR):
                    nc.vector.scalar_tensor_tensor(
                        out=wm_v[:], in0=kown, scalar=float(i), in1=dl[:],
                        op0=Op.is_le, op1=Op.mult,
                        accum_out=accV[:, i:i + 1])
                for i in range(S_THR):
                    nc.scalar.activation(
                        out=wm_s[:], in_=et[:], func=Act.Relu,
                        bias=biast[:, i:i + 1], scale=-1.0,
                        accum_out=accSA[:, i:i + 1])
                    nc.scalar.activation(
                        out=wm_s[:], in_=kown, func=Act.Relu,
                        bias=biast[:, i:i + 1], scale=-1.0,
                        accum_out=accSB[:, i:i + 1])

                # --- combine accumulators -> M[128, 32], col s = thr s ---
                # scalar-engine cols: ans = 16*(sum relu(m-k) - sum relu(m-e))
                M = ap.tile([128, STEPS], F32, tag="M")
                nc.vector.tensor_copy(out=M[:, 0:V_THR], in_=accV[:])
                nc.vector.tensor_tensor(
                    out=M[:, V_THR:STEPS], in0=accSB[:], in1=accSA[:],
                    op=Op.subtract)
                nc.vector.tensor_scalar(
                    out=M[:, V_THR:STEPS], in0=M[:, V_THR:STEPS],
                    scalar1=16.0, scalar2=0.0, op0=Op.mult, op1=Op.add)

                # --- partition partials -> per-image curves (PSUM accumulate) ---
                nc.tensor.matmul(
                    psum[:], bdt[:, c * IMGS:(c + 1) * IMGS], M[:],
                    start=(c == 0), stop=(c == NCHUNK - 1))

            outt = cst.tile([IMGS, STEPS], F32)
            nc.vector.tensor_copy(out=outt[:], in_=psum[:])
            nc.sync.dma_start(out=out[:], in_=outt[:])

    nc.finalize()
    return nc


def _bd_host():
    bd = np.zeros((128, NCHUNK * IMGS), dtype=np.float32)
    for c in range(NCHUNK):
        for p in range(128):
            bd[p, c * IMGS + c * CHUNK_IMGS + p // RB] = 1.0
    return bd


def kernel(x: np.ndarray) -> np.ndarray:
    assert x.shape == (B, C, H, W) and x.dtype == np.float32
    if "nc" not in _NC_CACHE:
        _NC_CACHE["nc"] = _build_nc()
    nc = _NC_CACHE["nc"]

    bd = _bd_host()
    in_maps = []
    for i in range(NCORES):
        shard = x[i * (B // NCORES):(i + 1) * (B // NCORES)]  # (8, 3, 256, 256)
        in_maps.append({
            "x": np.ascontiguousarray(shard).reshape(NCHUNK * 128, FD),
            "bd": bd,
        })
    res = run_bass_kernel_spmd(nc, in_maps, core_ids=list(range(NCORES)))
    parts = [res.results[i]["out"].reshape(B // NCORES, C, STEPS)
             for i in range(NCORES)]
    return np.concatenate(parts, axis=0).reshape(B, C * STEPS).astype(np.float32)


if __name__ == "__main__":
    rng = np.random.default_rng(0)
    x = rng.random((B, C, H, W), dtype=np.float32)
    y = kernel(x)
    print("kernel out", y.shape, y.dtype, y[:2, :6])


# revision 8
# speedup vs baseline: 1.1773x; 1.1773x over previous
"""Trainium2 Bass kernel for nn_CubECLayr: Euler characteristic curves of
sublevel cubical complexes, batch-data-parallel over 8 NeuronCores.

Algorithm (per core, 24 images of 256x256):
  1. k = ceil(x / DT) per pixel (exact integer bin in fp16), via fused
     multiply + magic-number round on the vector engine.
  2. Vertex attribution: every cell (vertex/edge/square) of the cubical
     complex is anchored to its (value, index)-max vertex; the signed count
     of cells anchored at each pixel is an integer delta in [-3, 3] computed
     from neighbor comparisons in k-space.  Then
         ECC_s = sum_p delta_p * [k_p <= s]
     which is exactly V - E + Sq of the sublevel complex at threshold s.
  3. The 32 threshold reductions are split across TWO engines:
     - Vector (DVE): direct fused compare-multiply-accumulate
       (scalar_tensor_tensor) -> sum(delta * [k <= s]) in one pass.
     - Scalar (ACT): encode e = k + delta/16 (exact in f16).  For
       m = s + 0.5:  sum(relu(m-k)) - sum(relu(m-e)) = (1/16)*sum(delta*
       [k<=s]), each sum one activation+accumulate pass.
     A block-diagonal ones matmul reduces partitions -> per-image curves.

Layout: 3 chunks x 8 images; each image owns 16 partitions (16 rows each,
one halo row above/below via SBUF shift-DMAs, pad=1000 at image borders).
"""

import numpy as np

import concourse.bacc as bacc
import concourse.mybir as mybir
from concourse import tile
from concourse.bass_utils import run_bass_kernel_spmd

NCORES = 8
B, C, H, W = 64, 3, 256, 256
IMGS = (B // NCORES) * C          # 24 images per core
CHUNK_IMGS = 8
NCHUNK = IMGS // CHUNK_IMGS       # 3
RB = 16                           # partitions per image
ROWS = H // RB                    # 16 own rows per partition
FD = ROWS * W                     # 4096 own pixels per partition
STEPS = 32
PAD = 1000.0                      # > any real bin; exact in fp16
MAGIC = 8388608.0                 # 2^23
HALF = float(np.float32(0.49999997))
F32 = mybir.dt.float32
F16 = mybir.dt.float16
Op = mybir.AluOpType
Act = mybir.ActivationFunctionType

# threshold split: vector does s in [0, V_THR) via one-pass STT; the scalar
# engine does s in [V_THR, 32) via relu-pair activations on (kt, et).
V_THR = 18
S_THR = STEPS - V_THR

_NC_CACHE = {}


def _build_nc():
    nc = bacc.Bacc(None, target_bir_lowering=False)
    x_in = nc.dram_tensor("x", [NCHUNK * 128, FD], F32, kind="ExternalInput")
    bd_in = nc.dram_tensor("bd", [128, NCHUNK * IMGS], F32, kind="ExternalInput")
    out = nc.dram_tensor("out", [IMGS, STEPS], F32, kind="ExternalOutput")

    HW2 = (ROWS + 2) * W          # kt/khe width (own + 2 halo rows)
    HW1 = (ROWS + 1) * W          # rv/ut width

    with tile.TileContext(nc) as tc:
        with (
            tc.tile_pool(name="xp", bufs=2) as xp,
            tc.tile_pool(name="wp", bufs=1) as wp,
            tc.tile_pool(name="wp2", bufs=2) as wp2,
            tc.tile_pool(name="ap", bufs=2) as ap,
            tc.tile_pool(name="cst", bufs=1) as cst,
            tc.tile_pool(name="pp", bufs=1, space="PSUM") as pp,
        ):
            bdt = cst.tile([128, NCHUNK * IMGS], F32)
            nc.sync.dma_start(out=bdt[:], in_=bd_in[:])
            padt = cst.tile([CHUNK_IMGS, W], F16)
            nc.vector.memset(padt[:], PAD)
            # per-threshold bias values m = s + 0.5 for the scalar engine
            biast = cst.tile([128, S_THR], F32)
            for i in range(S_THR):
                nc.gpsimd.memset(biast[:, i:i + 1], V_THR + i + 0.5)
            psum = pp.tile([IMGS, STEPS], F32)

            for c in range(NCHUNK):
                xt = xp.tile([128, FD], F32, tag="xt")
                nc.sync.dma_start(out=xt[:], in_=x_in[c * 128:(c + 1) * 128, :])

                # --- bins: k = round(x*31 + (0.5 - eps)) == ceil(x/DT) ---
                nc.vector.tensor_scalar(
                    out=xt[:], in0=xt[:], scalar1=31.0, scalar2=HALF,
                    op0=Op.mult, op1=Op.add)
                # kt rows: 0 = top halo, 1..16 own, 17 = bottom halo (flat cols)
                kt = wp2.tile([128, HW2], F16, tag="kt")
                nc.vector.tensor_scalar(
                    out=kt[:, W:W + FD], in0=xt[:], scalar1=MAGIC, scalar2=-MAGIC,
                    op0=Op.add, op1=Op.add)
                # halo exchange between partitions (same image), pad at borders
                nc.vector.memset(kt[:, 0:W], PAD)
                nc.vector.memset(kt[:, FD + W:FD + 2 * W], PAD)
                nc.sync.dma_start(out=kt[1:128, 0:W], in_=kt[0:127, FD:FD + W])
                nc.sync.dma_start(out=kt[0:127, FD + W:FD + 2 * W],
                                  in_=kt[1:128, W:2 * W])
                ktop = kt[:, 0:W].rearrange("(a b) w -> a b w", b=RB)
                nc.sync.dma_start(out=ktop[:, 0, :], in_=padt[:])
                kbot = kt[:, FD + W:FD + 2 * W].rearrange("(a b) w -> a b w", b=RB)
                nc.sync.dma_start(out=kbot[:, RB - 1, :], in_=padt[:])
                kown = kt[:, W:W + FD]

                # --- neighbor comparisons (k-space) ---
                # rh[r, j] = [k(r, j+1) >= k(r, j)], own rows, j = 0..254
                # (col 255 crosses rows; harmless, later masked via t zeroing)
                rh = wp.tile([128, FD], F16, tag="rh")
                nc.vector.tensor_tensor(
                    out=rh[:], in0=kt[:, W + 1:W + FD + 1], in1=kown,
                    op=Op.is_ge)
                # rv[t, j] = [k(row t+1) >= k(row t)], t = 0..16 (17 rows)
                rv = wp.tile([128, HW1], F16, tag="rv")
                nc.vector.tensor_tensor(
                    out=rv[:], in0=kt[:, W:], in1=kt[:, 0:HW1], op=Op.is_ge)
                # khe[r, j] = max(k(r, j), k(r, j+1)), rows 0..17
                khe = wp.tile([128, HW2], F16, tag="khe")
                nc.vector.tensor_tensor(
                    out=khe[:, 0:HW2 - 1], in0=kt[:, 0:HW2 - 1],
                    in1=kt[:, 1:HW2], op=Op.max)
                nc.vector.memset(khe[:, HW2 - 1:HW2], PAD)
                # u[t, j] = [khe(row t+1, j) >= khe(row t, j)], t = 0..16
                ut = wp.tile([128, HW1], F16, tag="ut")
                nc.vector.tensor_tensor(
                    out=ut[:], in0=khe[:, W:], in1=khe[:, 0:HW1], op=Op.is_ge)
                # Cc[r, j] = u(r) - u(r-1) for own rows r (u rows 1..16 - 0..15)
                cc = wp.tile([128, FD], F16, tag="cc")
                nc.vector.tensor_tensor(
                    out=cc[:], in0=ut[:, W:], in1=ut[:, 0:FD], op=Op.subtract)
                # zero col 255 of each row (cross-row garbage in rh/cc)
                cc3 = cc[:].rearrange("p (r w) -> p r w", w=W)
                nc.vector.memset(cc3[:, :, W - 1:W], 0.0)
                # t = rh * Cc
                tt = wp.tile([128, FD], F16, tag="tt")
                nc.vector.tensor_tensor(out=tt[:], in0=rh[:], in1=cc[:], op=Op.mult)

                # --- delta assembly ---
                # delta = rv(below) - rv(above) + t - shift1(t) - Cc
                dl = wp2.tile([128, FD], F16, tag="dl")
                nc.vector.tensor_tensor(
                    out=dl[:], in0=rv[:, W:], in1=rv[:, 0:FD], op=Op.subtract)
                nc.vector.tensor_tensor(out=dl[:], in0=dl[:], in1=tt[:], op=Op.add)
                nc.vector.tensor_tensor(
                    out=dl[:], in0=dl[:], in1=cc[:], op=Op.subtract)
                nc.vector.tensor_tensor(
                    out=dl[:, 1:FD], in0=dl[:, 1:FD], in1=tt[:, 0:FD - 1],
                    op=Op.subtract)

                # --- e = k + delta/16 (exact in f16), for the scalar engine ---
                dl16 = wp.tile([128, FD], F16, tag="dl16")
                nc.vector.tensor_scalar(
                    out=dl16[:], in0=dl[:], scalar1=1.0 / 16.0, scalar2=0.0,
                    op0=Op.mult, op1=Op.add)
                et = wp2.tile([128, FD], F16, tag="et")
                nc.vector.tensor_tensor(out=et[:], in0=kown, in1=dl16[:], op=Op.add)

                # --- 32 threshold reductions, split across 2 engines ---
                accV = ap.tile([128, V_THR], F32, tag="accv")
                accSA = ap.tile([128, S_THR], F32, tag="accsa")
                accSB = ap.tile([128, S_THR], F32, tag="accsb")
                wm_v = wp.tile([128, FD], F16, tag="wmv")
                wm_s = wp.tile([128, FD], F16, tag="wms")
                for i in range(V_THR):
                    nc.vector.scalar_tensor_tensor(
                        out=wm_v[:], in0=kown, scalar=float(i), in1=dl[:],
                        op0=Op.is_le, op1=Op.mult,
                        accum_out=accV[:, i:i + 1])
                for i in range(S_THR):
                    nc.scalar.activation(
                        out=wm_s[:], in_=et[:], func=Act.Relu,
                        bias=biast[:, i:i + 1], scale=-1.0,
                        accum_out=accSA[:, i:i + 1])
                    nc.scalar.activation(
                        out=wm_s[:], in_=kown, func=Act.Relu,
                        bias=biast[:, i:i + 1], scale=-1.0,
                        accum_out=accSB[:, i:i + 1])

                # --- combine accumulators -> M[128, 32], col s = thr s ---
                # scalar-engine cols: ans = 16*(sum relu(m-k) - sum relu(m-e))
                M = ap.tile([128, STEPS], F32, tag="M")
                nc.vector.tensor_copy(out=M[:, 0:V_THR], in_=accV[:])
                nc.vector.tensor_tensor(
                    out=M[:, V_THR:STEPS], in0=accSB[:], in1=accSA[:],
                    op=Op.subtract)
                nc.vector.tensor_scalar(
                    out=M[:, V_THR:STEPS], in0=M[:, V_THR:STEPS],
                    scalar1=16.0, scalar2=0.0, op0=Op.mult, op1=Op.add)

                # --- partition partials -> per-image curves (PSUM accumulate) ---
                nc.tensor.matmul(
                    psum[:], bdt[:, c * IMGS:(c + 1) * IMGS], M[:],
                    start=(c == 0), stop=(c == NCHUNK - 1))

            outt = cst.tile([IMGS, STEPS], F32)
            nc.vector.tensor_copy(out=outt[:], in_=psum[:])
            nc.sync.dma_start(out=out[:], in_=outt[:])

    nc.finalize()
    return nc


def _bd_host():
    bd = np.zeros((128, NCHUNK * IMGS), dtype=np.float32)
    for c in range(NCHUNK):
        for p in range(128):
            bd[p, c * IMGS + c * CHUNK_IMGS + p // RB] = 1.0
    return bd


def kernel(x: np.ndarray) -> np.ndarray:
    assert x.shape == (B, C, H, W) and x.dtype == np.float32
    if "nc" not in _NC_CACHE:
        _NC_CACHE["nc"] = _build_nc()
    nc = _NC_CACHE["nc"]

    bd = _bd_host()
    in_maps = []
    for i in range(NCORES):
        shard = x[i * (B // NCORES):(i + 1) * (B // NCORES)]  # (8, 3, 256, 256)
        in_maps.append({
            "x": np.ascontiguousarray(shard).reshape(NCHUNK * 128, FD),
            "bd": bd,
        })
    res = run_bass_kernel_spmd(nc, in_maps, core_ids=list(range(NCORES)))
    parts = [res.results[i]["out"].reshape(B // NCORES, C, STEPS)
             for i in range(NCORES)]
    return np.concatenate(parts, axis=0).reshape(B, C * STEPS).astype(np.float32)


if __name__ == "__main__":
    rng = np.random.default_rng(0)
    x = rng.random((B, C, H, W), dtype=np.float32)
    y = kernel(x)
    print("kernel out", y.shape, y.dtype, y[:2, :6])


# revision 12
# speedup vs baseline: 1.2883x; 1.0943x over previous
"""Trainium2 Bass kernel for nn_CubECLayr: Euler characteristic curves of
sublevel cubical complexes, batch-data-parallel over 8 NeuronCores.

Algorithm (per core, 24 images of 256x256):
  1. k = ceil(x / DT) per pixel (exact integer bin in fp16), via fused
     multiply + magic-number round on the vector engine.
  2. Vertex attribution: every cell (vertex/edge/square) of the cubical
     complex is anchored to its (value, index)-max vertex; the signed count
     of cells anchored at each pixel is an integer delta in [-3, 3] computed
     from neighbor comparisons in k-space.  Then
         ECC_s = sum_p delta_p * [k_p <= s]
     which is exactly V - E + Sq of the sublevel complex at threshold s.
  3. The 32 threshold reductions are split across TWO engines:
     - Vector (DVE): direct fused compare-multiply-accumulate
       (scalar_tensor_tensor) -> sum(delta * [k <= s]) in one pass.
     - Scalar (ACT): encode e = k + delta/16 (exact in f16).  For
       m = s + 0.5:  sum(relu(m-k)) - sum(relu(m-e)) = (1/16)*sum(delta*
       [k<=s]), each sum one activation+accumulate pass.
     A block-diagonal ones matmul reduces partitions -> per-image curves.

Layout: 3 chunks x 8 images; each image owns 16 partitions (16 rows each,
one halo row above/below via SBUF shift-DMAs, pad=1000 at image borders).
"""

import numpy as np

import concourse.bacc as bacc
import concourse.mybir as mybir
from concourse import tile
from concourse.bass_utils import run_bass_kernel_spmd

NCORES = 8
B, C, H, W = 64, 3, 256, 256
IMGS = (B // NCORES) * C          # 24 images per core
CHUNK_IMGS = 8
NCHUNK = IMGS // CHUNK_IMGS       # 3
RB = 16                           # partitions per image
ROWS = H // RB                    # 16 own rows per partition
FD = ROWS * W                     # 4096 own pixels per partition
STEPS = 32
PAD = 1000.0                      # > any real bin; exact in fp16
MAGIC = 8388608.0                 # 2^23
HALF = float(np.float32(0.49999997))
F32 = mybir.dt.float32
F16 = mybir.dt.float16
Op = mybir.AluOpType
Act = mybir.ActivationFunctionType

# threshold split: vector does s in [0, V_THR) via one-pass STT; the scalar
# engine does s in [V_THR, 32) via relu-pair activations on (kt, et).
V_THR = 18
S_THR = STEPS - V_THR

_NC_CACHE = {}


def _build_nc():
    nc = bacc.Bacc(None, target_bir_lowering=False)
    x_in = nc.dram_tensor("x", [NCHUNK * 128, FD], F32, kind="ExternalInput")
    bd_in = nc.dram_tensor("bd", [128, NCHUNK * IMGS], F32, kind="ExternalInput")
    out = nc.dram_tensor("out", [IMGS, STEPS], F32, kind="ExternalOutput")

    HW2 = (ROWS + 2) * W          # kt/khe width (own + 2 halo rows)
    HW1 = (ROWS + 1) * W          # rv/ut width

    with tile.TileContext(nc) as tc:
        with (
            tc.tile_pool(name="xp", bufs=2) as xp,
            tc.tile_pool(name="wp", bufs=1) as wp,
            tc.tile_pool(name="wp2", bufs=2) as wp2,
            tc.tile_pool(name="ap", bufs=2) as ap,
            tc.tile_pool(name="cst", bufs=1) as cst,
            tc.tile_pool(name="pp", bufs=1, space="PSUM") as pp,
        ):
            bdt = cst.tile([128, NCHUNK * IMGS], F32)
            nc.sync.dma_start(out=bdt[:], in_=bd_in[:])
            padt = cst.tile([CHUNK_IMGS, W], F16)
            nc.vector.memset(padt[:], PAD)
            # per-threshold bias values m = s + 0.5 for the scalar engine
            biast = cst.tile([128, S_THR], F32)
            for i in range(S_THR):
                nc.gpsimd.memset(biast[:, i:i + 1], V_THR + i + 0.5)
            psum = pp.tile([IMGS, STEPS], F32)

            for c in range(NCHUNK):
                xt = xp.tile([128, FD], F32, tag="xt")
                nc.sync.dma_start(out=xt[:], in_=x_in[c * 128:(c + 1) * 128, :])

                # --- bins: k = round(x*31 + (0.5 - eps)) == ceil(x/DT) ---
                nc.vector.tensor_scalar(
                    out=xt[:], in0=xt[:], scalar1=31.0, scalar2=HALF,
                    op0=Op.mult, op1=Op.add)
                # kt rows: 0 = top halo, 1..16 own, 17 = bottom halo (flat cols)
                kt = wp.tile([128, HW2], F16, tag="kt")
                nc.vector.tensor_scalar(
                    out=kt[:, W:W + FD], in0=xt[:], scalar1=MAGIC, scalar2=-MAGIC,
                    op0=Op.add, op1=Op.add)
                # halo exchange between partitions (same image), pad at borders
                nc.vector.memset(kt[:, 0:W], PAD)
                nc.vector.memset(kt[:, FD + W:FD + 2 * W], PAD)
                nc.sync.dma_start(out=kt[1:128, 0:W], in_=kt[0:127, FD:FD + W])
                nc.sync.dma_start(out=kt[0:127, FD + W:FD + 2 * W],
                                  in_=kt[1:128, W:2 * W])
                ktop = kt[:, 0:W].rearrange("(a b) w -> a b w", b=RB)
                nc.sync.dma_start(out=ktop[:, 0, :], in_=padt[:])
                kbot = kt[:, FD + W:FD + 2 * W].rearrange("(a b) w -> a b w", b=RB)
                nc.sync.dma_start(out=kbot[:, RB - 1, :], in_=padt[:])
                kown = kt[:, W:W + FD]

                # scalar-engine k-side passes: only need kt, start early
                accSB = ap.tile([128, S_THR], F32, tag="accsb")
                wm_s = wp.tile([128, FD], F16, tag="wms")
                for i in range(S_THR):
                    nc.scalar.activation(
                        out=wm_s[:], in_=kown, func=Act.Relu,
                        bias=biast[:, i:i + 1], scale=-1.0,
                        accum_out=accSB[:, i:i + 1])

                # --- neighbor comparisons (k-space) ---
                # rh[r, j] = [k(r, j+1) >= k(r, j)], own rows, j = 0..254
                # (col 255 crosses rows; harmless, later masked via t zeroing)
                rh = wp.tile([128, FD], F16, tag="rh")
                nc.vector.memset(rh[:, FD - 1:FD], 0.0)
                nc.vector.tensor_tensor(
                    out=rh[:, 0:FD - 1], in0=kt[:, W + 1:W + FD],
                    in1=kt[:, W:W + FD - 1], op=Op.is_ge)
                # rv[t, j] = [k(row t+1) >= k(row t)], t = 0..16 (17 rows)
                rv = wp.tile([128, HW1], F16, tag="rv")
                nc.vector.tensor_tensor(
                    out=rv[:], in0=kt[:, W:], in1=kt[:, 0:HW1], op=Op.is_ge)
                # khe[r, j] = max(k(r, j), k(r, j+1)), rows 0..17
                khe = wp.tile([128, HW2], F16, tag="khe")
                nc.vector.tensor_tensor(
                    out=khe[:, 0:HW2 - 1], in0=kt[:, 0:HW2 - 1],
                    in1=kt[:, 1:HW2], op=Op.max)
                nc.vector.memset(khe[:, HW2 - 1:HW2], PAD)
                # u[t, j] = [khe(row t+1, j) >= khe(row t, j)], t = 0..16
                ut = wp.tile([128, HW1], F16, tag="ut")
                nc.vector.tensor_tensor(
                    out=ut[:], in0=khe[:, W:], in1=khe[:, 0:HW1], op=Op.is_ge)
                # Cc[r, j] = u(r) - u(r-1) for own rows r (u rows 1..16 - 0..15)
                cc = wp.tile([128, FD], F16, tag="cc")
                nc.vector.tensor_tensor(
                    out=cc[:], in0=ut[:, W:], in1=ut[:, 0:FD], op=Op.subtract)
                # zero col 255 of each row (cross-row garbage in rh/cc)
                cc3 = cc[:].rearrange("p (r w) -> p r w", w=W)
                nc.vector.memset(cc3[:, :, W - 1:W], 0.0)
                # t = rh * Cc
                tt = wp.tile([128, FD], F16, tag="tt")
                nc.vector.tensor_tensor(out=tt[:], in0=rh[:], in1=cc[:], op=Op.mult)

                # --- delta assembly ---
                # delta = rv(below) - rv(above) + t - shift1(t) - Cc
                dl = wp2.tile([128, FD], F16, tag="dl")
                nc.vector.tensor_tensor(
                    out=dl[:], in0=rv[:, W:], in1=rv[:, 0:FD], op=Op.subtract)
                nc.vector.tensor_tensor(out=dl[:], in0=dl[:], in1=tt[:], op=Op.add)
                nc.vector.tensor_tensor(
                    out=dl[:], in0=dl[:], in1=cc[:], op=Op.subtract)
                nc.vector.tensor_tensor(
                    out=dl[:, 1:FD], in0=dl[:, 1:FD], in1=tt[:, 0:FD - 1],
                    op=Op.subtract)

                # --- e = k + delta/16 (exact in f16), for the scalar engine ---
                dl16 = wp.tile([128, FD], F16, tag="dl16")
                nc.vector.tensor_scalar(
                    out=dl16[:], in0=dl[:], scalar1=1.0 / 16.0, scalar2=0.0,
                    op0=Op.mult, op1=Op.add)
                et = wp2.tile([128, FD], F16, tag="et")
                nc.vector.tensor_tensor(out=et[:], in0=kown, in1=dl16[:], op=Op.add)

                # --- 32 threshold reductions, split across 2 engines ---
                accV = ap.tile([128, V_THR], F32, tag="accv")
                accSA = ap.tile([128, S_THR], F32, tag="accsa")
                wm_v = wp.tile([128, FD], F16, tag="wmv")
                for i in range(S_THR):
                    nc.scalar.activation(
                        out=wm_s[:], in_=et[:], func=Act.Relu,
                        bias=biast[:, i:i + 1], scale=-1.0,
                        accum_out=accSA[:, i:i + 1])
                for i in range(V_THR):
                    nc.vector.scalar_tensor_tensor(
                        out=wm_v[:], in0=kown, scalar=float(i), in1=dl[:],
                        op0=Op.is_le, op1=Op.mult,
                        accum_out=accV[:, i:i + 1])

                # --- combine accumulators -> M[128, 32], col s = thr s ---
                # scalar-engine cols: ans = 16*(sum relu(m-k) - sum relu(m-e))
                M = ap.tile([128, STEPS], F32, tag="M")
                nc.vector.tensor_copy(out=M[:, 0:V_THR], in_=accV[:])
                nc.vector.tensor_tensor(
                    out=M[:, V_THR:STEPS], in0=accSB[:], in1=accSA[:],
                    op=Op.subtract)
                nc.vector.tensor_scalar(
                    out=M[:, V_THR:STEPS], in0=M[:, V_THR:STEPS],
                    scalar1=16.0, scalar2=0.0, op0=Op.mult, op1=Op.add)

                # --- partition partials -> per-image curves (PSUM accumulate) ---
                nc.tensor.matmul(
                    psum[:], bdt[:, c * IMGS:(c + 1) * IMGS], M[:],
                    start=(c == 0), stop=(c == NCHUNK - 1))

            outt = cst.tile([IMGS, STEPS], F32)
            nc.vector.tensor_copy(out=outt[:], in_=psum[:])
            nc.sync.dma_start(out=out[:], in_=outt[:])

    nc.finalize()
    return nc


def _bd_host():
    bd = np.zeros((128, NCHUNK * IMGS), dtype=np.float32)
    for c in range(NCHUNK):
        for p in range(128):
            bd[p, c * IMGS + c * CHUNK_IMGS + p // RB] = 1.0
    return bd


def kernel(x: np.ndarray) -> np.ndarray:
    assert x.shape == (B, C, H, W) and x.dtype == np.float32
    if "nc" not in _NC_CACHE:
        _NC_CACHE["nc"] = _build_nc()
    nc = _NC_CACHE["nc"]

    bd = _bd_host()
    in_maps = []
    for i in range(NCORES):
        shard = x[i * (B // NCORES):(i + 1) * (B // NCORES)]  # (8, 3, 256, 256)
        in_maps.append({
            "x": np.ascontiguousarray(shard).reshape(NCHUNK * 128, FD),
            "bd": bd,
        })
    res = run_bass_kernel_spmd(nc, in_maps, core_ids=list(range(NCORES)))
    parts = [res.results[i]["out"].reshape(B // NCORES, C, STEPS)
             for i in range(NCORES)]
    return np.concatenate(parts, axis=0).reshape(B, C * STEPS).astype(np.float32)


if __name__ == "__main__":
    rng = np.random.default_rng(0)
    x = rng.random((B, C, H, W), dtype=np.float32)
    y = kernel(x)
    print("kernel out", y.shape, y.dtype, y[:2, :6])
